# revision 16
# baseline (speedup 1.0000x reference)
"""Pixel-contrastive loss on 8 Trainium2 NeuronCores (Bass/Tile).

Math (the same-label mask is all-ones whenever every label is equal; if any
two labels differ the reference loss contains -log(0) = +inf, handled on
host):

    f    : (N, C, H, W) viewed column-major as fT [C, M], M = N*H*W
    g_m  = f_m / max(||f_m||, 1e-12)                  (unit rows)
    logits_ij = 10 * (g_i . g_j)                      (temperature 0.1)
    denom_i   = sum_j exp(logits_ij) + 1e-6
    loss = mean_ij [ log(denom_i) - logits_ij ]
         = (1/M) sum_i log(denom_i) - (10/M^2) * ||sum_j g_j||^2

Sharding: row-block SPMD. Every core receives the full fT panel (8 MB) plus
its own 1024-column row panel; it normalizes locally (no collectives) and
computes its 1024xM block of logits, exp+row-sum fused on the scalar engine.
Host combines 8 small [128, *] outputs into the scalar loss.
"""

import sys

import numpy as np

for _p in ("/opt/trn_rl_repo",):
    if _p not in sys.path:
        sys.path.insert(0, _p)

TEMPERATURE_INV = 10.0
DENOM_EPS = 1e-6

# matmul operand dtype: "f32r" (full-rate fp32), "bf16", or "f32" (1/4 rate)
_MM_DTYPE = "bf16"

_PROGRAM_CACHE = {}
_LAST_RESULTS = None  # stashed BassKernelResults for test harness inspection


def _build_program(M, C, R, mm_dtype):
    """Bass/Tile program for one core: R-row block of an MxM masked softmax
    denominator + log, plus the s = sum_j g_j vector.

    Inputs : fall [C, M] fp32 (full normalized-feature source panel)
             frows [C, R] fp32 (this core's row panel, a column slice of fall)
    Outputs: out_ld [128, R//128] fp32  (log denominators, chunk-major)
             out_s  [128, 2] fp32      (s vector: channels 0-127, 128-255)
    """
    import concourse.mybir as mybir
    import concourse.tile as tile
    from concourse import bacc

    # Keep Exp and Ln in one activation-table set: the table-load pass picks
    # the first set containing each function, so by default Exp->set 0 and
    # Ln->set 5 thrash ~25 ACT_TABLE_LOADs (~32us). Restricting both
    # functions to 'natural_log_exp_and_others' (indices unchanged - only
    # other sets' function lists shrink, so act_func_set_id stays valid)
    # yields a single load.
    if not getattr(bacc, "_act_tables_pinned_exp_ln", False):
        _orig_get_tables = bacc.get_activation_tables

        def _pinned_tables(arch):
            tables = _orig_get_tables(arch)
            exp_ln = {mybir.ActivationFunctionType.Exp,
                      mybir.ActivationFunctionType.Ln}
            for name, funcs in tables.items():
                if name != "natural_log_exp_and_others" and exp_ln & funcs:
                    tables[name] = funcs - exp_ln
            return tables

        bacc.get_activation_tables = _pinned_tables
        bacc._act_tables_pinned_exp_ln = True

    f32 = mybir.dt.float32
    AF = mybir.ActivationFunctionType
    ALU = mybir.AluOpType

    KC = C // 128          # contraction chunks (2)
    RCH = R // 128         # row chunks per core (8)
    BLK = 2048             # column block: 4 PSUM banks
    NBLK = M // BLK
    NT = BLK // 512        # matmul slices per block

    mmdt = {
        "f32r": mybir.dt.float32r,
        "bf16": mybir.dt.bfloat16,
        "f32": f32,
    }[mm_dtype]
    g_dt = mmdt
    nrm_dt = f32 if mm_dtype == "f32" else mybir.dt.float32r

    nc = bacc.Bacc("TRN2", target_bir_lowering=False, debug=False)
    fall = nc.dram_tensor("fall", [C, M], f32, kind="ExternalInput")
    frows = nc.dram_tensor("frows", [C, R], f32, kind="ExternalInput")
    out_ld = nc.dram_tensor("out_ld", [128, RCH], f32, kind="ExternalOutput")
    out_s = nc.dram_tensor("out_s", [128, 2], f32, kind="ExternalOutput")

    with tile.TileContext(nc) as tc:
        with (
            tc.tile_pool(name="consts", bufs=1) as consts,
            tc.tile_pool(name="fpool", bufs=1) as fpool,
            tc.tile_pool(name="gpool", bufs=1) as gpool,
            tc.tile_pool(name="sqp", bufs=2) as sqp,
            tc.tile_pool(name="stats", bufs=1) as stats,
            tc.tile_pool(name="raccp", bufs=2) as raccp,
            tc.tile_pool(name="psum", bufs=2, space="PSUM") as psum,
        ):
            ones_f32 = consts.tile([128, 128], f32)
            nc.vector.memset(ones_f32, 1.0)
            ones = consts.tile([128, 128], nrm_dt)
            nc.vector.tensor_copy(ones, ones_f32)
            epst = consts.tile([128, 1], f32)
            nc.vector.memset(epst, DENOM_EPS)
            zerot = consts.tile([128, 1], f32)
            nc.vector.memset(zerot, 0.0)
            # ACT bounce scratch: in-place PSUM->PSUM activations run at half
            # rate (single-ported banks), so activations bounce through SBUF.
            # Write-only / ACT-serial, so one buffer suffices.
            et = consts.tile([128, BLK], f32, name="et")

            def normalize(dst, src, width, sacc=None):
                """dst[k] = unit-normalized src[k] columns (k = C chunk).

                Column norms^2 come from an all-ones matmul (broadcast across
                partitions), rsqrt = exp(-0.5 * ln(x)) stays in one ACT table
                set. With sacc, also accumulates per-chunk column sums of dst
                into sacc[k] (one column per call) for the s vector.
                """
                sq = []
                for k in range(KC):
                    t = sqp.tile([128, BLK], nrm_dt, tag=f"sq{k}", name=f"sq{k}")
                    # squares on GpSimd (otherwise idle) so the DVE keeps the
                    # g = f * inv multiplies flowing during the ramp
                    nc.gpsimd.tensor_mul(t[:, :width], src[k], src[k])
                    sq.append(t)
                pt = psum.tile([128, BLK], f32, tag="pt", name="pt")
                for k in range(KC):
                    for s0 in range(0, width, 512):
                        sl = slice(s0, min(s0 + 512, width))
                        nc.tensor.matmul(
                            pt[:, sl],
                            ones,
                            sq[k][:, sl],
                            start=(k == 0),
                            stop=(k == KC - 1),
                        )
                nc.scalar.activation(
                    out=et[:, :width], in_=pt[:, :width], func=AF.Ln,
                    bias=zerot, scale=1.0,
                )
                nc.scalar.activation(
                    out=pt[:, :width], in_=et[:, :width], func=AF.Exp,
                    bias=zerot, scale=-0.5,
                )
                for k in range(KC):
                    if sacc is None:
                        nc.vector.tensor_mul(dst[k], src[k], pt[:, :width])
                    else:
                        # fused g = f * inv with accum_out = per-block column
                        # sums for the s vector (tensor_tensor_reduce crashes
                        # on this runtime; scalar_tensor_tensor accum works)
                        nc.vector.scalar_tensor_tensor(
                            out=dst[k], in0=src[k], scalar=1.0,
                            in1=pt[:, :width],
                            op0=ALU.mult, op1=ALU.mult,
                            accum_out=sacc[k],
                        )

            # --- row panel: load + normalize ---
            fr = [fpool.tile([128, R], f32, tag=f"fr{k}", name=f"fr{k}") for k in range(KC)]
            for k in range(KC):
                nc.sync.dma_start(out=fr[k], in_=frows[k * 128:(k + 1) * 128, :])
            gr = [gpool.tile([128, R], g_dt, tag=f"gr{k}", name=f"gr{k}") for k in range(KC)]
            normalize(gr, fr, R)

            # --- full panel: load + normalize blockwise, accumulate s ---
            f = [fpool.tile([128, M], f32, tag=f"f{k}", name=f"f{k}") for k in range(KC)]
            g = [gpool.tile([128, M], g_dt, tag=f"g{k}", name=f"g{k}") for k in range(KC)]
            sacc = [stats.tile([128, NBLK], f32, tag=f"sacc{k}", name=f"sacc{k}") for k in range(KC)]
            for b in range(NBLK):
                cs = slice(b * BLK, (b + 1) * BLK)
                for k in range(KC):
                    nc.sync.dma_start(
                        out=f[k][:, cs], in_=fall[k * 128:(k + 1) * 128, cs]
                    )
                normalize(
                    [g[k][:, cs] for k in range(KC)],
                    [f[k][:, cs] for k in range(KC)],
                    BLK,
                    sacc=[sacc[k][:, b:b + 1] for k in range(KC)],
                )

            # --- logits row blocks: matmul -> fused exp+rowsum -> log ---
            # Column-block OUTER so block b only needs g[:, b] — all 8 row
            # chunks of block 0 overlap with normalization of blocks 1..3.
            ldbuf = stats.tile([128, RCH], f32)
            raccs = [
                stats.tile([128, NBLK], f32, tag=f"racc{r}", name=f"racc{r}")
                for r in range(RCH)
            ]
            for b in range(NBLK):
                for r in range(RCH):
                    pt2 = psum.tile([128, BLK], f32, tag="pt", name="pt2")
                    for k in range(KC):
                        lhs = gr[k][:, r * 128:(r + 1) * 128]
                        for s in range(NT):
                            cs = slice(b * BLK + s * 512, b * BLK + (s + 1) * 512)
                            nc.tensor.matmul(
                                pt2[:, s * 512:(s + 1) * 512],
                                lhs,
                                g[k][:, cs],
                                start=(k == 0),
                                stop=(k == KC - 1),
                            )
                    nc.scalar.activation(
                        out=et, in_=pt2, func=AF.Exp,
                        bias=zerot, scale=TEMPERATURE_INV,
                        accum_out=raccs[r][:, b:b + 1],
                    )
            for r in range(RCH):
                rsum = raccp.tile([128, 1], f32, tag="rsum", name="rsum")
                nc.vector.tensor_reduce(
                    rsum, raccs[r], axis=mybir.AxisListType.X, op=ALU.add
                )
                nc.scalar.activation(
                    out=ldbuf[:, r:r + 1], in_=rsum, func=AF.Ln,
                    bias=epst, scale=1.0,
                )

            # --- epilogue: s vector + DMA out ---
            s_out = stats.tile([128, 2], f32)
            for k in range(KC):
                nc.vector.tensor_reduce(
                    s_out[:, k:k + 1], sacc[k], axis=mybir.AxisListType.X,
                    op=ALU.add,
                )
            nc.sync.dma_start(out=out_ld[:, :], in_=ldbuf)
            nc.sync.dma_start(out=out_s[:, :], in_=s_out)

    nc.finalize()
    return nc


def kernel(features, labels):
    global _LAST_RESULTS
    from concourse.bass_utils import run_bass_kernel_spmd

    features = np.ascontiguousarray(np.asarray(features), dtype=np.float32)
    labels_np = np.asarray(labels)
    lab = labels_np.reshape(-1)
    if lab.size and not bool(np.all(lab == lab[0])):
        # Any cross-label pair makes the reference loss -log(0) = +inf.
        return np.float32(np.inf)

    N, C, H, W = features.shape
    M = N * H * W
    n_cores = 8
    R = M // n_cores
    fT = np.ascontiguousarray(features.transpose(1, 0, 2, 3).reshape(C, M))

    key = (M, C, R, _MM_DTYPE)
    if key not in _PROGRAM_CACHE:
        _PROGRAM_CACHE[key] = _build_program(M, C, R, _MM_DTYPE)
    nc = _PROGRAM_CACHE[key]

    in_maps = [
        {"fall": fT, "frows": np.ascontiguousarray(fT[:, k * R:(k + 1) * R])}
        for k in range(n_cores)
    ]
    res = run_bass_kernel_spmd(nc, in_maps, list(range(n_cores)))
    _LAST_RESULTS = res
    results = res.results

    slog = sum(r["out_ld"].astype(np.float64).sum() for r in results)
    s_cols = results[0]["out_s"].astype(np.float64)
    s = np.concatenate([s_cols[:, 0], s_cols[:, 1]])
    sum_logits = TEMPERATURE_INV * float(s @ s)
    loss = slog / M - sum_logits / (float(M) * float(M))
    return np.float32(loss)


# revision 19
# speedup vs baseline: 1.2199x; 1.2199x over previous
"""Pixel-contrastive loss on 8 Trainium2 NeuronCores (Bass/Tile).

Math (the same-label mask is all-ones whenever every label is equal; if any
two labels differ the reference loss contains -log(0) = +inf, handled on
host):

    f    : (N, C, H, W) viewed column-major as fT [C, M], M = N*H*W
    g_m  = f_m / max(||f_m||, 1e-12)                  (unit rows)
    logits_ij = 10 * (g_i . g_j)                      (temperature 0.1)
    denom_i   = sum_j exp(logits_ij) + 1e-6
    loss = mean_ij [ log(denom_i) - logits_ij ]
         = (1/M) sum_i log(denom_i) - (10/M^2) * ||sum_j g_j||^2

Sharding: row-block SPMD. Every core receives the full fT panel (8 MB) plus
its own 1024-column row panel; it normalizes locally (no collectives) and
computes its 1024xM block of logits, exp+row-sum fused on the scalar engine.
Host combines 8 small [128, *] outputs into the scalar loss.
"""

import sys

import numpy as np

for _p in ("/opt/trn_rl_repo",):
    if _p not in sys.path:
        sys.path.insert(0, _p)

TEMPERATURE_INV = 10.0
DENOM_EPS = 1e-6

# matmul operand dtype: "f32r" (full-rate fp32), "bf16", or "f32" (1/4 rate)
_MM_DTYPE = "bf16"

_PROGRAM_CACHE = {}
_LAST_RESULTS = None  # stashed BassKernelResults for test harness inspection


def _build_program(M, C, R, mm_dtype):
    """Bass/Tile program for one core: R-row block of an MxM masked softmax
    denominator + log, plus the s = sum_j g_j vector.

    Inputs : fall [C, M] fp32 (full normalized-feature source panel)
             frows [C, R] fp32 (this core's row panel, a column slice of fall)
    Outputs: out_ld [128, R//128] fp32  (log denominators, chunk-major)
             out_s  [128, 2] fp32      (s vector: channels 0-127, 128-255)
    """
    import concourse.mybir as mybir
    import concourse.tile as tile
    from concourse import bacc

    # Keep Exp and Ln in one activation-table set: the table-load pass picks
    # the first set containing each function, so by default Exp->set 0 and
    # Ln->set 5 thrash ~25 ACT_TABLE_LOADs (~32us). Restricting both
    # functions to 'natural_log_exp_and_others' (indices unchanged - only
    # other sets' function lists shrink, so act_func_set_id stays valid)
    # yields a single load.
    if not getattr(bacc, "_act_tables_pinned_exp_ln", False):
        _orig_get_tables = bacc.get_activation_tables

        def _pinned_tables(arch):
            tables = _orig_get_tables(arch)
            exp_ln = {mybir.ActivationFunctionType.Exp,
                      mybir.ActivationFunctionType.Ln}
            for name, funcs in tables.items():
                if name != "natural_log_exp_and_others" and exp_ln & funcs:
                    tables[name] = funcs - exp_ln
            return tables

        bacc.get_activation_tables = _pinned_tables
        bacc._act_tables_pinned_exp_ln = True

    f32 = mybir.dt.float32
    AF = mybir.ActivationFunctionType
    ALU = mybir.AluOpType

    KC = C // 128          # contraction chunks (2)
    RCH = R // 128         # row chunks per core (8)
    BLK = 2048             # column block: 4 PSUM banks
    NBLK = M // BLK
    NT = BLK // 512        # matmul slices per block

    mmdt = {
        "f32r": mybir.dt.float32r,
        "bf16": mybir.dt.bfloat16,
        "f32": f32,
    }[mm_dtype]
    g_dt = mmdt
    nrm_dt = f32 if mm_dtype == "f32" else mybir.dt.float32r

    nc = bacc.Bacc("TRN2", target_bir_lowering=False, debug=False)
    fall = nc.dram_tensor("fall", [C, M], f32, kind="ExternalInput")
    frows = nc.dram_tensor("frows", [C, R], f32, kind="ExternalInput")
    out_ld = nc.dram_tensor("out_ld", [128, RCH], f32, kind="ExternalOutput")
    out_s = nc.dram_tensor("out_s", [128, 2], f32, kind="ExternalOutput")

    with tile.TileContext(nc) as tc:
        with (
            tc.tile_pool(name="consts", bufs=1) as consts,
            tc.tile_pool(name="fpool", bufs=1) as fpool,
            tc.tile_pool(name="gpool", bufs=1) as gpool,
            tc.tile_pool(name="sqp", bufs=2) as sqp,
            tc.tile_pool(name="invp", bufs=2) as invp,
            tc.tile_pool(name="stats", bufs=1) as stats,
            tc.tile_pool(name="raccp", bufs=2) as raccp,
            tc.tile_pool(name="psum", bufs=2, space="PSUM") as psum,
        ):
            ones_f32 = consts.tile([128, 128], f32)
            nc.vector.memset(ones_f32, 1.0)
            ones = consts.tile([128, 128], nrm_dt)
            nc.vector.tensor_copy(ones, ones_f32)
            epst = consts.tile([128, 1], f32)
            nc.vector.memset(epst, DENOM_EPS)
            zerot = consts.tile([128, 1], f32)
            nc.vector.memset(zerot, 0.0)
            # ACT bounce scratch: in-place PSUM->PSUM activations run at half
            # rate (single-ported banks), so activations bounce through SBUF.
            # Write-only / ACT-serial, so one buffer suffices.
            et = consts.tile([128, BLK], f32, name="et")

            def normalize(dst, src, width, sacc=None):
                """dst[k] = unit-normalized src[k] columns (k = C chunk).

                Column norms^2 come from an all-ones matmul (broadcast across
                partitions), rsqrt = exp(-0.5 * ln(x)) stays in one ACT table
                set. With sacc, also accumulates per-chunk column sums of dst
                into sacc[k] (one column per call) for the s vector.
                """
                sq = []
                for k in range(KC):
                    t = sqp.tile([128, BLK], nrm_dt, tag=f"sq{k}", name=f"sq{k}")
                    nc.vector.tensor_mul(t[:, :width], src[k], src[k])
                    sq.append(t)
                pt = psum.tile([128, BLK], f32, tag="pt", name="pt")
                for k in range(KC):
                    for s0 in range(0, width, 512):
                        sl = slice(s0, min(s0 + 512, width))
                        nc.tensor.matmul(
                            pt[:, sl],
                            ones,
                            sq[k][:, sl],
                            start=(k == 0),
                            stop=(k == KC - 1),
                        )
                # Ln frees the PSUM slot immediately (phase-2 logits blocks
                # contend for the same pool); Exp lands the inverse norms in
                # SBUF for the DVE multiplies.
                nc.scalar.activation(
                    out=et[:, :width], in_=pt[:, :width], func=AF.Ln,
                    bias=zerot, scale=1.0,
                )
                inv = invp.tile([128, BLK], f32, tag="inv", name="inv")
                nc.scalar.activation(
                    out=inv[:, :width], in_=et[:, :width], func=AF.Exp,
                    bias=zerot, scale=-0.5,
                )
                for k in range(KC):
                    if sacc is None:
                        nc.vector.tensor_mul(dst[k], src[k], inv[:, :width])
                    else:
                        # fused g = f * inv with accum_out = per-block column
                        # sums for the s vector (tensor_tensor_reduce crashes
                        # on this runtime; scalar_tensor_tensor accum works)
                        nc.vector.scalar_tensor_tensor(
                            out=dst[k], in0=src[k], scalar=1.0,
                            in1=inv[:, :width],
                            op0=ALU.mult, op1=ALU.mult,
                            accum_out=sacc[k],
                        )

            # --- row panel: load + normalize ---
            fr = [fpool.tile([128, R], f32, tag=f"fr{k}", name=f"fr{k}") for k in range(KC)]
            for k in range(KC):
                nc.sync.dma_start(out=fr[k], in_=frows[k * 128:(k + 1) * 128, :])
            gr = [gpool.tile([128, R], g_dt, tag=f"gr{k}", name=f"gr{k}") for k in range(KC)]
            normalize(gr, fr, R)

            # --- full panel: load + normalize blockwise, accumulate s ---
            f = [fpool.tile([128, M], f32, tag=f"f{k}", name=f"f{k}") for k in range(KC)]
            g = [gpool.tile([128, M], g_dt, tag=f"g{k}", name=f"g{k}") for k in range(KC)]
            sacc = [stats.tile([128, NBLK], f32, tag=f"sacc{k}", name=f"sacc{k}") for k in range(KC)]
            for b in range(NBLK):
                cs = slice(b * BLK, (b + 1) * BLK)
                for k in range(KC):
                    nc.sync.dma_start(
                        out=f[k][:, cs], in_=fall[k * 128:(k + 1) * 128, cs]
                    )
                normalize(
                    [g[k][:, cs] for k in range(KC)],
                    [f[k][:, cs] for k in range(KC)],
                    BLK,
                    sacc=[sacc[k][:, b:b + 1] for k in range(KC)],
                )

            # --- logits row blocks: matmul -> fused exp+rowsum -> log ---
            # Column-block OUTER so block b only needs g[:, b] — all 8 row
            # chunks of block 0 overlap with normalization of blocks 1..3.
            ldbuf = stats.tile([128, RCH], f32)
            raccs = [
                stats.tile([128, NBLK], f32, tag=f"racc{r}", name=f"racc{r}")
                for r in range(RCH)
            ]
            for b in range(NBLK):
                for r in range(RCH):
                    pt2 = psum.tile([128, BLK], f32, tag="pt", name="pt2")
                    for k in range(KC):
                        lhs = gr[k][:, r * 128:(r + 1) * 128]
                        for s in range(NT):
                            cs = slice(b * BLK + s * 512, b * BLK + (s + 1) * 512)
                            nc.tensor.matmul(
                                pt2[:, s * 512:(s + 1) * 512],
                                lhs,
                                g[k][:, cs],
                                start=(k == 0),
                                stop=(k == KC - 1),
                            )
                    nc.scalar.activation(
                        out=et, in_=pt2, func=AF.Exp,
                        bias=zerot, scale=TEMPERATURE_INV,
                        accum_out=raccs[r][:, b:b + 1],
                    )
            for r in range(RCH):
                rsum = raccp.tile([128, 1], f32, tag="rsum", name="rsum")
                nc.vector.tensor_reduce(
                    rsum, raccs[r], axis=mybir.AxisListType.X, op=ALU.add
                )
                nc.scalar.activation(
                    out=ldbuf[:, r:r + 1], in_=rsum, func=AF.Ln,
                    bias=epst, scale=1.0,
                )

            # --- epilogue: s vector + DMA out ---
            s_out = stats.tile([128, 2], f32)
            for k in range(KC):
                nc.vector.tensor_reduce(
                    s_out[:, k:k + 1], sacc[k], axis=mybir.AxisListType.X,
                    op=ALU.add,
                )
            nc.sync.dma_start(out=out_ld[:, :], in_=ldbuf)
            nc.sync.dma_start(out=out_s[:, :], in_=s_out)

    nc.finalize()
    return nc


def kernel(features, labels):
    global _LAST_RESULTS
    from concourse.bass_utils import run_bass_kernel_spmd

    features = np.ascontiguousarray(np.asarray(features), dtype=np.float32)
    labels_np = np.asarray(labels)
    lab = labels_np.reshape(-1)
    if lab.size and not bool(np.all(lab == lab[0])):
        # Any cross-label pair makes the reference loss -log(0) = +inf.
        return np.float32(np.inf)

    N, C, H, W = features.shape
    M = N * H * W
    n_cores = 8
    R = M // n_cores
    fT = np.ascontiguousarray(features.transpose(1, 0, 2, 3).reshape(C, M))

    key = (M, C, R, _MM_DTYPE)
    if key not in _PROGRAM_CACHE:
        _PROGRAM_CACHE[key] = _build_program(M, C, R, _MM_DTYPE)
    nc = _PROGRAM_CACHE[key]

    in_maps = [
        {"fall": fT, "frows": np.ascontiguousarray(fT[:, k * R:(k + 1) * R])}
        for k in range(n_cores)
    ]
    res = run_bass_kernel_spmd(nc, in_maps, list(range(n_cores)))
    _LAST_RESULTS = res
    results = res.results

    slog = sum(r["out_ld"].astype(np.float64).sum() for r in results)
    s_cols = results[0]["out_s"].astype(np.float64)
    s = np.concatenate([s_cols[:, 0], s_cols[:, 1]])
    sum_logits = TEMPERATURE_INV * float(s @ s)
    loss = slog / M - sum_logits / (float(M) * float(M))
    return np.float32(loss)
